# revision 19
# baseline (speedup 1.0000x reference)
"""Trainium2 Bass kernel for nn_Cov_EBFLayer.

Math: out[b,o] = exp(-quad[o,b]),
  quad[o,b] = diff^T P_o diff,  diff = c_o - x_b,  P_o = B_o B_o^T
            = x^T P x - 2 v_o^T x + q3_o,   v = P c,  q3 = c^T P c
Square trick (kills the bcast*x elementwise multiply):
  x^T P x = 0.5 sum_{d,f} P[d,f] (x_d + x_f)^2 - sum_d r_d x_d^2,  r = P 1
so with W = 0.5 P (folded as betas * sqrt(0.5)):
  quad = sum_{d,f} W[d,f] (x_d+x_f)^2  - r.x^2 - 2 v.x + q3

Kernel per core (batch-sharded 8 x 1024):
  - warmup matmuls from a memset tile (PE p-state ramp, no DMA dependency)
  - Gram: P_o = B_o^T B_o for o-pairs (ol, ol+64), PSUM partitions (q, d),
    copies (DVE/ACT alternating, 2-group batches) -> p_sb2 [(q,d), (h,f,ttl)]
  - DRAM round trip per o-half re-reads P as W chunks [(j,f), (c, o)]
    (256B contiguous runs both ways, no o permutation anywhere)
  - builds: indicator matmul with two 1s per column -> (x_d + x_f) in PSUM,
    Square on ACT/DVE -> gstore fp16; mains: 32 accumulating matmuls + aug
    chunk [x; x^2] with coeffs [-2v; -r]; q3 via per-partition Exp bias.
  - inputs land via 3 parallel DMA queues (sync/scalar/vector).
Host does layout-only prep + tiny linear-term prep (w,v,q3,r,s: ~3M MACs
= 0.02% of model FLOPs).
"""

import sys
from contextlib import ExitStack

import numpy as np

sys.path.insert(0, "/opt/trn_rl_repo")

import concourse.bass as bass  # noqa: E402
import concourse.tile as tile  # noqa: E402
from concourse import bacc, mybir  # noqa: E402
from concourse import bass_utils  # noqa: E402
from concourse._compat import with_exitstack  # noqa: E402

B, D, O, NCORES = 8192, 64, 256, 8
BSH = B // NCORES  # 1024 per-core batch shard
NC = D // 2  # 32 quadratic chunks, each (2 d's) x (64 f's) = 128 partitions
BT = 512  # b-tile (one PSUM bank of fp32)
NBT = BSH // BT  # 2
F32 = mybir.dt.float32
F16 = mybir.dt.float16
COPY = mybir.ActivationFunctionType.Copy
SQUARE = mybir.ActivationFunctionType.Square
EXP = mybir.ActivationFunctionType.Exp


@with_exitstack
def _kernel(ctx: ExitStack, tc, outT, xT, bt2h0, bt2h1, indc2, waug1, q3b):
    nc = tc.nc

    cpool = ctx.enter_context(tc.tile_pool(name="const", bufs=1))
    ppool = ctx.enter_context(tc.tile_pool(name="psum_p", bufs=3, space="PSUM"))
    qpool = ctx.enter_context(tc.tile_pool(name="psum_q", bufs=2, space="PSUM"))
    dpool = ctx.enter_context(tc.tile_pool(name="dram", bufs=1, space="DRAM"))

    # ---- SBUF residents ----
    warm = cpool.tile([D, BT], F16)  # warmup scratch (memset, no DMA dep)
    sb_bt2 = cpool.tile([128, 64 * 128], F16)  # [(h,e), (ttl, oo, d)]
    sb_indc2 = cpool.tile([D, NC * 128], F16)  # two-ones indicator
    aug1 = cpool.tile([128, BSH], F16)  # [xT; x^2]
    sb_waug1 = cpool.tile([128, O], F16)  # [-2 v^T; -r^T]
    sb_q3b = cpool.tile([128, 2], F32)  # -q3 per (o-half) column
    p_sb2 = cpool.tile([128, 2 * D * 64], F16)  # [(q,d), (h, f, ttl)]
    w = [cpool.tile([128, NC * 128], F16, name=f"w{h}") for h in range(2)]
    gstore = cpool.tile([128, NC * NBT * BT], F16)  # squared features
    osb = [cpool.tile([128, BSH], F16, name=f"osb{h}") for h in range(2)]
    p_dram = dpool.tile([2, 2, D, D, 64], F16)  # [h, q, d, f, tt]

    # ---- parallel input DMA: 3 queues ----
    # betasT2 quarters interleaved across sync+scalar so half 0 streams in
    # on two queues at once (Gram h0 is the critical consumer)
    for qx, (eng, h) in enumerate(
        [(nc.sync, 0), (nc.scalar, 0), (nc.sync, 0), (nc.scalar, 0)]
    ):
        sl = slice(qx * 2048, (qx + 1) * 2048)
        eng.dma_start(sb_bt2[0:64, sl], bt2h0[:, sl])
    for qx, (eng, h) in enumerate(
        [(nc.sync, 1), (nc.scalar, 1), (nc.sync, 1), (nc.scalar, 1)]
    ):
        sl = slice(qx * 2048, (qx + 1) * 2048)
        eng.dma_start(sb_bt2[64:128, sl], bt2h1[:, sl])
    nc.vector.memset(warm[:], 0.125)
    # gpsimd (SWDGE) queue: x, indicator, aug coeffs, bias
    nc.gpsimd.dma_start(aug1[0:D, :], xT[:])
    nc.gpsimd.dma_start(sb_indc2[:], indc2[:])
    nc.gpsimd.dma_start(sb_waug1[:], waug1[:])
    nc.gpsimd.dma_start(sb_q3b[:], q3b[:])

    # PSUM: ppool = 3 x [128, 1024] units (Gram + build tiles, 6 banks),
    # qpool = 2 banks (oh1 accumulators recycle oh0's after the early exp)
    pq = {}
    for oh in range(2):
        for bt in range(NBT):
            pq[(oh, bt)] = qpool.tile(
                [128, BT], F32, name=f"pq_{oh}_{bt}", tag="pq"
            )

    # ---- PE warmup: p-state ramp while input DMAs fly ----
    wps = ppool.tile([128, NBT * BT], F32, name="wps", tag="u")
    for i in range(7):
        nc.tensor.matmul(
            wps[:, 0:BT], warm[:, 0:128], warm[:], start=True, stop=True
        )

    # ---- Gram: pair ttl of half h covers o = h*128 + {ttl, 64+ttl} ----
    # PSUM partitions (q, d); 8 pairs per 2-bank unit tile; lo block copied
    # on DVE and hi block on ACT in parallel, contiguous-dst iteration.
    for h in range(2):
        dv = p_sb2[:, h * 4096 : (h + 1) * 4096].rearrange(
            "p (f tt) -> p f tt", f=64
        )
        for u in range(8):
            pt = ppool.tile(
                [128, NBT * BT], F32, name=f"gm_{h}_{u}", tag="u"
            )
            for t in range(8):
                ttl = u * 8 + t
                bsl = sb_bt2[h * 64 : h * 64 + 64, ttl * 128 : ttl * 128 + 128]
                nc.tensor.matmul(
                    pt[:, t * 128 : (t + 1) * 128], bsl, bsl, start=True, stop=True
                )
            sv = pt[:].rearrange("p (t oo f) -> p oo f t", t=8, oo=2)
            ts0 = u * 8
            nc.vector.tensor_copy(dv[0:64, :, ts0 : ts0 + 8], sv[0:64, 0])
            nc.scalar.activation(
                dv[64:128, :, ts0 : ts0 + 8], sv[64:128, 1], COPY
            )
        # ---- DRAM round trip for this half (sync queue) ----
        # p_dram[d, f, o] with o = h*128 + q*64 + ttl
        # h0 round trip on sync queue, h1 on scalar queue (parallel chains)
        deng = nc.sync if h == 0 else nc.scalar
        wr = p_dram[:].rearrange("hh q d f tt -> hh (q d) f tt")
        src = p_sb2[:, h * 4096 : (h + 1) * 4096].rearrange(
            "p (f tt) -> p f tt", f=64
        )
        deng.dma_start(wr[h], src)
        rd = p_dram[:].rearrange("hh q (c jj) f tt -> hh q (jj f) c tt", jj=2)
        wv = w[h][:].rearrange("p (c o) -> p c o", c=NC)
        for q in range(2):
            deng.dma_start(wv[:, :, q * 64 : (q + 1) * 64], rd[h, q])

    # ---- x^2 rows of aug chunk (ACT; after xT lands) ----
    for bt in range(NBT):
        nc.scalar.activation(
            aug1[64:128, bt * BT : (bt + 1) * BT],
            aug1[0:64, bt * BT : (bt + 1) * BT],
            SQUARE,
        )

    # ---- main phase A: builds + squares + oh0 mains ----
    # squares at [128, 1024] granularity (both b-tiles); PSUM can only feed
    # one operand, so 2/3 go ACT-Square direct, 1/3 via DVE copy + Pool mul.
    # mains trail builds by 2 chunks so the PE queue never head-blocks on a
    # square still in flight (3 units of PSUM in the 6-bank gt tile).
    stg = [cpool.tile([128, NBT * BT], F16, name=f"stg{i}") for i in range(2)]
    for cc in range(NC + 2):
        if cc < NC:
            c = cc
            bd = ppool.tile([128, NBT * BT], F32, name=f"bd_{c}", tag="u")
            for bt in range(NBT):
                nc.tensor.matmul(
                    bd[:, bt * BT : (bt + 1) * BT],
                    sb_indc2[:, c * 128 : (c + 1) * 128],
                    aug1[0:D, bt * BT : (bt + 1) * BT],
                    start=True,
                    stop=True,
                )
            gsl = gstore[:, c * NBT * BT : (c + 1) * NBT * BT]
            if c % 3 != 2:
                nc.scalar.activation(gsl, bd[:], SQUARE)
            else:
                st = stg[(c // 3) % 2]
                nc.vector.tensor_copy(st[:], bd[:])
                nc.gpsimd.tensor_mul(gsl, st[:], st[:])
        if cc >= 2:
            c = cc - 2
            for bt in range(NBT):
                nc.tensor.matmul(
                    pq[(0, bt)],
                    w[0][:, c * 128 : (c + 1) * 128],
                    gstore[:, (c * NBT + bt) * BT : (c * NBT + bt + 1) * BT],
                    start=(c == 0),
                    stop=False,
                )
    # aug mains + epilogue for oh0 (early, overlaps phase B)
    for bt in range(NBT):
        nc.tensor.matmul(
            pq[(0, bt)],
            sb_waug1[:, 0:128],
            aug1[:, bt * BT : (bt + 1) * BT],
            start=False,
            stop=True,
        )
        nc.scalar.activation(
            osb[0][:, bt * BT : (bt + 1) * BT],
            pq[(0, bt)],
            EXP,
            bias=sb_q3b[:, 0:1],
            scale=-1.0,
        )
        nc.gpsimd.dma_start(
            outT[0:128, bt * BT : (bt + 1) * BT],
            osb[0][:, bt * BT : (bt + 1) * BT],
        )

    # ---- main phase B: oh1 mains from stored squares ----
    for c in range(NC):
        for bt in range(NBT):
            nc.tensor.matmul(
                pq[(1, bt)],
                w[1][:, c * 128 : (c + 1) * 128],
                gstore[:, (c * NBT + bt) * BT : (c * NBT + bt + 1) * BT],
                start=(c == 0),
                stop=False,
            )
    for bt in range(NBT):
        nc.tensor.matmul(
            pq[(1, bt)],
            sb_waug1[:, 128:256],
            aug1[:, bt * BT : (bt + 1) * BT],
            start=False,
            stop=True,
        )
        nc.scalar.activation(
            osb[1][:, bt * BT : (bt + 1) * BT],
            pq[(1, bt)],
            EXP,
            bias=sb_q3b[:, 1:2],
            scale=-1.0,
        )
        nc.gpsimd.dma_start(
            outT[128:256, bt * BT : (bt + 1) * BT],
            osb[1][:, bt * BT : (bt + 1) * BT],
        )


_CACHE = {}


def _build():
    if "nc" in _CACHE:
        return _CACHE["nc"], _CACHE["aps"]
    nc = bacc.Bacc(
        "TRN2", target_bir_lowering=False, debug=False, num_devices=NCORES
    )
    xT = nc.dram_tensor("xT", [D, BSH], F16, kind="ExternalInput").ap()
    bt2h0 = nc.dram_tensor("bt2h0", [64, 8192], F16, kind="ExternalInput").ap()
    bt2h1 = nc.dram_tensor("bt2h1", [64, 8192], F16, kind="ExternalInput").ap()
    indc2 = nc.dram_tensor("indc2", [D, NC * 128], F16, kind="ExternalInput").ap()
    waug1 = nc.dram_tensor("waug1", [128, O], F16, kind="ExternalInput").ap()
    q3b = nc.dram_tensor("q3b", [128, 2], F32, kind="ExternalInput").ap()
    outT = nc.dram_tensor("outT", [O, BSH], F16, kind="ExternalOutput").ap()
    with tile.TileContext(nc) as tc:
        _kernel(tc, outT, xT, bt2h0, bt2h1, indc2, waug1, q3b)
    nc.compile()
    _CACHE["nc"] = nc
    _CACHE["aps"] = (xT, bt2h0, bt2h1, indc2, waug1, q3b, outT)
    return nc, _CACHE["aps"]


def _host_prep(x, centers, betas):
    x = np.asarray(x, np.float32)
    betas = np.asarray(betas, np.float32)
    c = np.asarray(centers, np.float32).reshape(O, D)
    # betasT2[h][e, (ttl, oo, d)] = betas[h*128+oo*64+ttl, d, e] * sqrt(1/2)
    bt = (betas * np.sqrt(0.5)).transpose(2, 0, 1)  # [e, o, d]
    bt = bt.reshape(D, 2, 2, 64, D)  # [e, h, oo, ttl, d]
    bt = bt.transpose(1, 0, 3, 2, 4).reshape(2, D, 8192)  # [h, e, (ttl,oo,d)]
    bt2h0 = np.ascontiguousarray(bt[0]).astype(np.float16)
    bt2h1 = np.ascontiguousarray(bt[1]).astype(np.float16)
    # two-ones indicator: indc2[k, c*128+p] = [k == 2c+p//64] + [k == p%64]
    k = np.arange(D)[:, None, None]
    cc = np.arange(NC)[None, :, None]
    p = np.arange(128)[None, None, :]
    indc2 = (k == 2 * cc + p // 64).astype(np.float32) + (k == p % 64)
    indc2 = np.ascontiguousarray(indc2.reshape(D, NC * 128)).astype(np.float16)
    # tiny linear-term prep: w = B^T c, v = B w, q3 = w.w, r = P 1 (~3M MACs)
    w_ = np.einsum("ofe,of->oe", betas, c)
    v = np.einsum("ode,oe->od", betas, w_)
    q3 = np.einsum("oe,oe->o", w_, w_)
    s = betas.sum(axis=1)  # [o, e]
    r = np.einsum("ode,oe->od", betas, s)
    waug1 = np.concatenate([-2.0 * v.T, -r.T], axis=0).astype(np.float16)
    q3b = np.ascontiguousarray((-q3).reshape(2, 128).T).astype(np.float32)
    xT_shards = [
        np.ascontiguousarray(x[i * BSH : (i + 1) * BSH].T).astype(np.float16)
        for i in range(NCORES)
    ]
    return xT_shards, bt2h0, bt2h1, indc2, waug1, q3b


def _run(x, centers, betas, trace=False):
    nc, (xT, bt2h0a, bt2h1a, indc2a, waug1a, q3ba, outT) = _build()
    xT_shards, bt2h0, bt2h1, indc2, waug1, q3b = _host_prep(x, centers, betas)
    in_maps = [
        {
            xT.name: xT_shards[i],
            bt2h0a.name: bt2h0,
            bt2h1a.name: bt2h1,
            indc2a.name: indc2,
            waug1a.name: waug1,
            q3ba.name: q3b,
        }
        for i in range(NCORES)
    ]
    res = bass_utils.run_bass_kernel_spmd(
        nc, in_maps, core_ids=list(range(NCORES)), trace=trace
    )
    out = np.concatenate(
        [np.asarray(res.results[i][outT.name]).T for i in range(NCORES)],
        axis=0,
    )
    return out.astype(np.float32), res


def kernel(x, centers, betas):
    out, _ = _run(x, centers, betas, trace=False)
    return out
